# revision 11
# baseline (speedup 1.0000x reference)
"""Channel-wise min/max stats kernel for Trainium2 (8 NeuronCores).

Input:  tensor [1024, 32768] float32
Output: (min_vals [1024], max_vals [1024]) float32  -- per-channel min/max

Sharding: channel axis split across 8 cores (128 channels each -> exactly the
128 SBUF partitions). Each core reduces its own rows; host concatenates.
No collectives needed.

Per-core kernel (raw Bass, manual sems -- the Tile framework's exit barrier
and the custom-DVE/TTR ISA ops don't compile on this toolchain): the 16 MiB
slice is streamed in N_CHUNKS chunk DMAs into one resident SBUF buffer. Each
chunk [128, CHUNK] is reduced with tensor_tensor_scan over its two halves:
    state = min(min(lo[t], state), hi[t])
which consumes 2 stream elements per cycle -- twice the rate of a plain
tensor_reduce. The scan state chains across chunks via `initial`, so the last
chunk's last output column IS the final per-channel stat. One scan chain for
min, one for max, ping-pong scratch buffers.
"""

import sys
from contextlib import ExitStack

for _p in ("/opt/trn_rl_repo",):
    if _p not in sys.path:
        sys.path.insert(0, _p)

import numpy as np

import concourse.bass as bass
import concourse.mybir as mybir
from concourse.bass_utils import run_bass_kernel_spmd

P = 128            # partitions = channels per core
W = 32768          # elements per channel
C = 1024           # total channels
N_CORES = 8
N_CHUNKS = 8
CHUNK = W // N_CHUNKS      # 4096
HALF = CHUNK // 2          # 2048
FMAX = 3.4028235e38

_NC_CACHE = {}


def _build_bass(sem_chain=False, detect_races=False):
    """Build the per-core program.

    sem_chain=True threads a semaphore through the DVE ops so CoreSim's race
    detector can verify the DMA<->DVE synchronization (the scratch reuse and
    scan-state chaining between back-to-back DVE ops is safe on HW -- the DVE
    executes in order and drains between ops -- but the detector can't know
    that). The production build omits the chain.
    """
    f32 = mybir.dt.float32
    nc = bass.Bass(detect_race_conditions=detect_races)
    x = nc.declare_dram_parameter("x", [P, W], f32, isOutput=False)
    mn_out = nc.declare_dram_parameter("mn", [P, 1], f32, isOutput=True)
    mx_out = nc.declare_dram_parameter("mx", [P, 1], f32, isOutput=True)

    with ExitStack() as ctx:
        data = ctx.enter_context(nc.sbuf_tensor([P, W], f32))
        scr_mn = [
            ctx.enter_context(nc.sbuf_tensor(f"scr_mn{i}", [P, HALF], f32))
            for i in range(2)
        ]
        scr_mx = [
            ctx.enter_context(nc.sbuf_tensor(f"scr_mx{i}", [P, HALF], f32))
            for i in range(2)
        ]
        ld_sems = [
            ctx.enter_context(nc.semaphore(f"ld{j}")) for j in range(N_CHUNKS)
        ]
        sem_v = ctx.enter_context(nc.semaphore("vec_done"))
        sem_st = ctx.enter_context(nc.semaphore("st_done"))
        sem_ch = (
            ctx.enter_context(nc.semaphore("dve_chain")) if sem_chain else None
        )
        block = ctx.enter_context(nc.Block())

        last = (N_CHUNKS - 1) % 2

        @block.sync
        def _(sync):
            for j in range(N_CHUNKS):
                sl = slice(j * CHUNK, (j + 1) * CHUNK)
                sync.dma_start(out=data[:, sl], in_=x[:, sl]).then_inc(
                    ld_sems[j], 16
                )
            if sem_chain:
                sync.wait_ge(sem_ch, 2 * N_CHUNKS)
            else:
                sync.wait_ge(sem_v, 1)
            sync.dma_start(
                out=mn_out[:], in_=scr_mn[last][:, HALF - 1 : HALF]
            ).then_inc(sem_st, 16)
            sync.dma_start(
                out=mx_out[:], in_=scr_mx[last][:, HALF - 1 : HALF]
            ).then_inc(sem_st, 16)
            sync.wait_ge(sem_st, 32)

        @block.vector
        def _(vector):
            k = 0
            for j in range(N_CHUNKS):
                lo = slice(j * CHUNK, j * CHUNK + HALF)
                hi = slice(j * CHUNK + HALF, (j + 1) * CHUNK)
                vector.wait_ge(ld_sems[j], 16)
                for scr, op, seed in (
                    (scr_mn, mybir.AluOpType.min, FMAX),
                    (scr_mx, mybir.AluOpType.max, -FMAX),
                ):
                    if sem_chain and k > 0:
                        vector.wait_ge(sem_ch, k)
                    init = (
                        seed if j == 0
                        else scr[(j - 1) % 2][:, HALF - 1 : HALF]
                    )
                    ins = nc.vector.tensor_tensor_scan(
                        out=scr[j % 2][:],
                        data0=data[:, lo],
                        data1=data[:, hi],
                        initial=init,
                        op0=op,
                        op1=op,
                    )
                    if sem_chain:
                        ins.then_inc(sem_ch, 1)
                    k += 1
            if not sem_chain:
                ins.then_inc(sem_v, 1)

    return nc


def _get_nc():
    if "nc" not in _NC_CACHE:
        _NC_CACHE["nc"] = _build_bass()
    return _NC_CACHE["nc"]


def run(tensor, trace=False):
    """Run the SPMD kernel; returns (min_vals, max_vals, BassKernelResults)."""
    x = np.ascontiguousarray(np.asarray(tensor, dtype=np.float32))
    assert x.shape == (C, W), x.shape
    in_maps = [
        {"x": np.ascontiguousarray(x[i * P : (i + 1) * P])} for i in range(N_CORES)
    ]
    nc = _get_nc()
    out = run_bass_kernel_spmd(nc, in_maps, core_ids=list(range(N_CORES)), trace=trace)
    mins = np.concatenate([r["mn"].reshape(P) for r in out.results])
    maxs = np.concatenate([r["mx"].reshape(P) for r in out.results])
    return mins, maxs, out


def kernel(tensor):
    mins, maxs, _ = run(tensor, trace=False)
    return mins, maxs


# revision 13
# speedup vs baseline: 1.1494x; 1.1494x over previous
"""Channel-wise min/max stats kernel for Trainium2 (8 NeuronCores).

Input:  tensor [1024, 32768] float32
Output: (min_vals [1024], max_vals [1024]) float32  -- per-channel min/max

Sharding: channel axis split across 8 cores (128 channels each -> exactly the
128 SBUF partitions). Each core reduces its own rows; host concatenates.
No collectives needed.

Per-core kernel (raw Bass, manual sems): the 16 MiB slice is streamed in
N_CHUNKS chunk DMAs into one resident SBUF buffer. Each chunk [128, CHUNK]
gets a tensor_reduce(min) and a tensor_reduce(max) into per-chunk partials
(DVE ingests 1 elem/cycle; min+max = 2 full passes = the DVE floor on this
toolchain -- fused 2-stream reduce ops don't compile and no other engine can
reduce along the free axis). Final tiny reduces collapse partials to [128,2],
one DMA out.
"""

import sys
from contextlib import ExitStack

for _p in ("/opt/trn_rl_repo",):
    if _p not in sys.path:
        sys.path.insert(0, _p)

import numpy as np

import concourse.bass as bass
import concourse.mybir as mybir
from concourse.bass_utils import run_bass_kernel_spmd

P = 128            # partitions = channels per core
W = 32768          # elements per channel
C = 1024           # total channels
N_CORES = 8
N_CHUNKS = 16
CHUNK = W // N_CHUNKS      # 2048
FMAX = 3.4028235e38

_NC_CACHE = {}


def _build_bass(sem_chain=False, detect_races=False):
    """Build the per-core program.

    sem_chain=True threads a semaphore through the DVE ops so CoreSim's race
    detector can verify the DMA<->DVE synchronization (the partials reuse
    between back-to-back DVE ops is safe on HW -- the DVE executes in order --
    but the detector can't know that). The production build omits the chain.
    """
    f32 = mybir.dt.float32
    nc = bass.Bass(detect_race_conditions=detect_races)
    x = nc.declare_dram_parameter("x", [P, W], f32, isOutput=False)
    mnmx_out = nc.declare_dram_parameter("mnmx", [P, 2], f32, isOutput=True)

    with ExitStack() as ctx:
        data = ctx.enter_context(nc.sbuf_tensor("data", [P, W], f32))
        mins = ctx.enter_context(nc.sbuf_tensor("mins", [P, N_CHUNKS], f32))
        maxs = ctx.enter_context(nc.sbuf_tensor("maxs", [P, N_CHUNKS], f32))
        mnmx = ctx.enter_context(nc.sbuf_tensor("mnmx_sb", [P, 2], f32))
        ld_sems = [
            ctx.enter_context(nc.semaphore(f"ld{j}")) for j in range(N_CHUNKS)
        ]
        sem_v = ctx.enter_context(nc.semaphore("vec_done"))
        sem_st = ctx.enter_context(nc.semaphore("st_done"))
        sem_ch = (
            ctx.enter_context(nc.semaphore("dve_chain")) if sem_chain else None
        )
        block = ctx.enter_context(nc.Block())

        @block.sync
        def _(sync):
            for j in range(N_CHUNKS):
                sl = slice(j * CHUNK, (j + 1) * CHUNK)
                sync.dma_start(out=data[:, sl], in_=x[:, sl]).then_inc(
                    ld_sems[j], 16
                )
            if sem_chain:
                sync.wait_ge(sem_ch, 2 * N_CHUNKS + 2)
            else:
                sync.wait_ge(sem_v, 1)
            sync.dma_start(out=mnmx_out[:], in_=mnmx[:]).then_inc(sem_st, 16)
            sync.wait_ge(sem_st, 16)

        @block.vector
        def _(vector):
            k = 0

            def chain_pre(vec):
                nonlocal k
                if sem_chain and k > 0:
                    vec.wait_ge(sem_ch, k)

            def chain_post(ins):
                nonlocal k
                if sem_chain:
                    ins.then_inc(sem_ch, 1)
                k += 1
                return ins

            for j in range(N_CHUNKS):
                sl = slice(j * CHUNK, (j + 1) * CHUNK)
                vector.wait_ge(ld_sems[j], 16)
                for op, dst in (
                    (mybir.AluOpType.min, mins),
                    (mybir.AluOpType.max, maxs),
                ):
                    chain_pre(vector)
                    chain_post(nc.vector.tensor_reduce(
                        out=dst[:, j : j + 1],
                        in_=data[:, sl],
                        axis=mybir.AxisListType.X,
                        op=op,
                    ))
            chain_pre(vector)
            chain_post(nc.vector.tensor_reduce(
                out=mnmx[:, 0:1], in_=mins[:], axis=mybir.AxisListType.X,
                op=mybir.AluOpType.min,
            ))
            chain_pre(vector)
            ins = nc.vector.tensor_reduce(
                out=mnmx[:, 1:2], in_=maxs[:], axis=mybir.AxisListType.X,
                op=mybir.AluOpType.max,
            )
            chain_post(ins)
            if not sem_chain:
                ins.then_inc(sem_v, 1)

    return nc


def _get_nc():
    if "nc" not in _NC_CACHE:
        _NC_CACHE["nc"] = _build_bass()
    return _NC_CACHE["nc"]


def run(tensor, trace=False):
    """Run the SPMD kernel; returns (min_vals, max_vals, BassKernelResults)."""
    x = np.ascontiguousarray(np.asarray(tensor, dtype=np.float32))
    assert x.shape == (C, W), x.shape
    in_maps = [
        {"x": np.ascontiguousarray(x[i * P : (i + 1) * P])} for i in range(N_CORES)
    ]
    nc = _get_nc()
    out = run_bass_kernel_spmd(nc, in_maps, core_ids=list(range(N_CORES)), trace=trace)
    mins = np.concatenate([r["mnmx"][:, 0] for r in out.results])
    maxs = np.concatenate([r["mnmx"][:, 1] for r in out.results])
    return mins, maxs, out


def kernel(tensor):
    mins, maxs, _ = run(tensor, trace=False)
    return mins, maxs
